# revision 30
# baseline (speedup 1.0000x reference)
"""Trainium2 Bass kernel: single-head causal attention.

B=4, T=4096, E=512, H=64, fp32 in/out.

Sharding: 2 cores per batch sample, split-K softmax (core 2b takes even
128-key strips, core 2b+1 odd strips, via the involutive half-rotation
of every 256-token block so the SPMD program is core-independent).
Host combines partials: out = (num0+num1)/(den0+den1).

v2 vs baseline:
- Scores matmuls are ROW-TILED 2x: contraction is H=64, so two key
  strips run concurrently in PE array rows 0:63 / 64:127, draining to
  two adjacent PSUM banks. Q^T is duplicated to partitions 64:128 via a
  [Wq|Wq] stationary (same moving-column cost), and odd strips' K^T is
  produced at partitions 64:128 via alternating [Wk|Wv] / [Wv|Wk]
  stationaries in the KV projection (same cost).
- One [128,1024] exp per strip-pair (both banks in one ACT instr), with
  the exp table preloaded behind the DMA phase by a tiny dummy exp.
- Causal mask: single fused [128,1024] DVE multiply on the diagonal pair.
- PV for the second diagonal strip streams only its live 256 columns.
- PE emission order: scores of pair i+1 before PV of pair i, with
  projection matmuls drained between pairs as filler; ~20 dummy identity
  matmuls bridge the DMA-bound lead-in so the PE HAM clock gate reaches
  2.4 GHz before the steady phase.
- KV projection accumulates both parity halves as ONE PSUM group:
  start=True clears has_written bank-wide on TRN2, so interleaved
  accumulation groups must not share a PSUM bank.
"""

import functools

import numpy as np
import ml_dtypes

B, T, E, H = 4, 4096, 512, 64
NCORES = 8
NCHUNK = 8  # 512-query chunks per sample
CHUNK = T // NCHUNK  # 512
NSTRIP = 16  # local 128-key strips per core (half of T/128)
VSTRIDE = 80  # per-strip stride in the packed V tile

bf16 = ml_dtypes.bfloat16


@functools.lru_cache(maxsize=1)
def _build():
    import concourse.mybir as mybir
    from concourse import bacc
    from concourse.masks import make_identity
    import concourse.tile as tile

    dt_bf = mybir.dt.bfloat16
    dt_f32 = mybir.dt.float32

    nc = bacc.Bacc("TRN2", target_bir_lowering=False, num_devices=NCORES)

    # x^T, rotated, (quarter, e-strip)-blocked:
    # [4 quarters, 128, 4 e-strips, 1024 tokens]
    xt = nc.dram_tensor("xt", [4, 128, 4, T // 4], dt_bf, kind="ExternalInput")
    wq = nc.dram_tensor("wq", [128, 4 * 128], dt_bf, kind="ExternalInput")
    wkva = nc.dram_tensor("wkva", [128, 4 * 128], dt_bf, kind="ExternalInput")
    wkvb = nc.dram_tensor("wkvb", [128, 4 * 128], dt_bf, kind="ExternalInput")
    bias_q = nc.dram_tensor("bias_q", [128, 1], dt_f32, kind="ExternalInput")
    bias_kv = nc.dram_tensor("bias_kv", [128, 2], dt_f32, kind="ExternalInput")
    masks = nc.dram_tensor("masks", [128, 2 * CHUNK], dt_bf, kind="ExternalInput")
    out_d = nc.dram_tensor("out", [H + 1, T], dt_f32, kind="ExternalOutput")
    import os
    DBG = os.environ.get("KDBG") == "1"
    if DBG:
        dbg_kv = nc.dram_tensor("dbg_kv", [128, 4 * 512], mybir.dt.bfloat16, kind="ExternalOutput")
        dbg_q = nc.dram_tensor("dbg_q", [128, 8 * 512], mybir.dt.bfloat16, kind="ExternalOutput")
        dbg_v = nc.dram_tensor("dbg_v", [128, NSTRIP * VSTRIDE], mybir.dt.bfloat16, kind="ExternalOutput")
        dbg_p = nc.dram_tensor("dbg_p", [128, 1024], mybir.dt.bfloat16, kind="ExternalOutput")
        dbg_s = nc.dram_tensor("dbg_s", [128, 1024], dt_f32, kind="ExternalOutput")

    with tile.TileContext(nc) as tc:
        with (
            tc.tile_pool(name="const", bufs=1) as cpool,
            tc.tile_pool(name="xt_pool", bufs=1) as xpool,
            tc.tile_pool(name="q_pool", bufs=NCHUNK) as qpool,
            tc.tile_pool(name="kv_pool", bufs=4) as kvpool,
            tc.tile_pool(name="v_pool", bufs=1) as vpool,
            tc.tile_pool(name="p_pool", bufs=4) as ppool,
            tc.tile_pool(name="o_pool", bufs=2) as opool,
            tc.tile_pool(name="ps_proj", bufs=2, space="PSUM") as pspr_pool,
            tc.tile_pool(name="ps_s", bufs=2, space="PSUM") as pss_pool,
            tc.tile_pool(name="ps_o", bufs=2, space="PSUM") as pso_pool,
        ):
            # ---- input DMAs: weights/biases first, then xt quarter 0
            # per e-strip (so kv0/q0 matmuls start ~1us in), masks, rest ----
            xt_sb = xpool.tile([128, 4 * T], dt_bf)
            wkva_sb = cpool.tile([128, 4 * 128], dt_bf)
            nc.sync.dma_start(wkva_sb, wkva.ap())
            wkvb_sb = cpool.tile([128, 4 * 128], dt_bf)
            nc.sync.dma_start(wkvb_sb, wkvb.ap())
            wq_sb = cpool.tile([128, 4 * 128], dt_bf)
            nc.sync.dma_start(wq_sb, wq.ap())
            bkv_sb = cpool.tile([128, 2], dt_f32)
            nc.sync.dma_start(bkv_sb, bias_kv.ap())
            bq_sb = cpool.tile([128, 1], dt_f32)
            nc.sync.dma_start(bq_sb, bias_q.ap())

            def xt_dma(qd, es):
                off = (qd * 4 + es) * 1024
                nc.sync.dma_start(
                    xt_sb[:, off : off + 1024], xt.ap()[qd][:, es, :]
                )

            for es in range(4):
                xt_dma(0, es)
            masks_sb = cpool.tile([128, 2 * CHUNK], dt_bf)
            nc.sync.dma_start(masks_sb, masks.ap())
            for qd in range(1, 4):
                for es in range(4):
                    xt_dma(qd, es)
            ident = cpool.tile([128, 128], dt_bf)
            make_identity(nc, ident)
            # HAM warm-up: ~20 dependency-free matmuls on the identity keep
            # the PE busy through the DMA phase so the clock reaches 2.4 GHz
            # before real work starts
            warm_ps = pss_pool.tile([128, 2 * CHUNK], dt_f32, tag="pss")
            for _ in range(32):
                nc.tensor.matmul(
                    warm_ps[:, 0:128], lhsT=ident, rhs=ident,
                    start=True, stop=True,
                )
            # preload the exp table set during the DMA/proj warmup
            warm = cpool.tile([128, 1], dt_f32)
            nc.scalar.activation(
                warm, ident[:, 0:1], mybir.ActivationFunctionType.Exp, scale=1.0
            )

            # packed V (natural [k,h] layout + ones column for denominator)
            v_nat = vpool.tile([128, NSTRIP * VSTRIDE], dt_bf)
            v3 = v_nat.rearrange("p (s c) -> p s c", c=VSTRIDE)
            nc.vector.memset(v3[:, :, 64:65], 1.0)

            def xt_block(qd, es):
                off = (qd * 4 + es) * 1024
                return xt_sb[:, off : off + 1024]

            scale = 1.0 / float(np.sqrt(H))
            kv_tiles = []
            q_tiles = []

            def kv_proj_pieces(ckv):
                # keys: first 128 tokens of each 256-block; even local
                # strips (a=0,2) with [Wk|Wv] -> K^T rows 0:64; odd strips
                # (a=1,3) with [Wv|Wk] -> K^T rows 64:128.
                state = {}

                def mk_mm(es):
                    def run():
                        if es == 0:
                            state["ps"] = pspr_pool.tile(
                                [128, CHUNK], dt_f32, tag="proj",
                                name=f"ps_kv{ckv}",
                            )
                        key_rhs = xt_block(ckv, es).rearrange(
                            "p (c d two b) -> p c d two b", d=2, two=2, b=128
                        )
                        # both parity halves live in ONE PSUM bank and
                        # start=True clears has_written bank-wide: only the
                        # very first matmul may start the group
                        for par, w_sb in ((0, wkva_sb), (1, wkvb_sb)):
                            nc.tensor.matmul(
                                state["ps"][:, par * 256 : (par + 1) * 256],
                                lhsT=w_sb[:, es * 128 : (es + 1) * 128],
                                rhs=key_rhs[:, :, par, 0, :],
                                start=(es == 0 and par == 0),
                                stop=(es == 3 and par == 1),
                                skip_group_check=True,
                            )

                    return run

                def fin():
                    kv_sb = kvpool.tile([128, CHUNK], dt_bf, tag="kv")
                    nc.vector.tensor_scalar_add(
                        kv_sb[:, 0:256], state["ps"][:, 0:256], bkv_sb[:, 0:1]
                    )
                    nc.vector.tensor_scalar_add(
                        kv_sb[:, 256:512], state["ps"][:, 256:512], bkv_sb[:, 1:2]
                    )
                    kv_tiles.append(kv_sb)
                    if DBG:
                        nc.sync.dma_start(
                            dbg_kv.ap()[:, ckv * 512 : (ckv + 1) * 512], kv_sb
                        )

                return [mk_mm(es) for es in range(4)] + [fin]

            def kv_proj(ckv):
                for piece in kv_proj_pieces(ckv):
                    piece()

            def strip_col(j):
                # parity-contiguous kv layout: strip j (0..3 within chunk)
                return (j % 2) * 256 + (j // 2) * 128

            def v_transpose(s):
                # strip s: V^T lives at rows 64:128 (even s) / 0:64 (odd s)
                # of the kv tile; PE transpose puts V at cols 64:128 / 0:64.
                kv_sb = kv_tiles[s // 4]
                col = strip_col(s % 4)
                ps_tr = pspr_pool.tile([128, 128], dt_bf, tag="proj")
                nc.tensor.transpose(ps_tr, kv_sb[:, col : col + 128], ident)
                vcols = ps_tr[:, 64:128] if s % 2 == 0 else ps_tr[:, 0:64]
                nc.vector.tensor_copy(
                    v_nat[:, s * VSTRIDE : s * VSTRIDE + 64], vcols
                )

            def q_proj_pieces(c):
                # [Wq|Wq] stationary -> Q^T duplicated on rows 0:64, 64:128
                state = {}

                def mk_mm(es):
                    def run():
                        if es == 0:
                            state["ps"] = pspr_pool.tile(
                                [128, CHUNK], dt_f32, tag="proj",
                                name=f"ps_q{c}",
                            )
                        nc.tensor.matmul(
                            state["ps"],
                            lhsT=wq_sb[:, es * 128 : (es + 1) * 128],
                            rhs=xt_block(c // 2, es)[
                                :, (c % 2) * CHUNK : (c % 2) * CHUNK + CHUNK
                            ],
                            start=(es == 0),
                            stop=(es == 3),
                        )

                    return run

                def fin():
                    q_sb = qpool.tile([128, CHUNK], dt_bf, tag="q")
                    nc.vector.tensor_scalar_add(q_sb, state["ps"], bq_sb)
                    q_tiles.append(q_sb)
                    if DBG:
                        nc.sync.dma_start(
                            dbg_q.ap()[:, c * 512 : (c + 1) * 512], q_sb
                        )

                return [mk_mm(es) for es in range(4)] + [fin]

            def q_proj(c):
                for piece in q_proj_pieces(c):
                    piece()

            def kt_ap(l):
                # K^T strip l as a [64,128] lhsT at row parity l%2
                kv_sb = kv_tiles[l // 4]
                rows = (0, 64) if l % 2 == 0 else (64, 128)
                col = strip_col(l % 4)
                return kv_sb[rows[0] : rows[1], col : col + 128]

            # ---- deferred-work queue: projection pieces emitted as PE
            # filler between attention pairs. Each group tracks a cursor so
            # ensure() can force-complete a group right before its consumer.
            class Group:
                def __init__(self, pieces):
                    self.pieces = pieces
                    self.i = 0

                def step(self):
                    if self.i < len(self.pieces):
                        self.pieces[self.i]()
                        self.i += 1
                        return True
                    return False

                def finish(self):
                    while self.step():
                        pass

            pending = []

            def drain(k):
                done = 0
                while done < k and pending:
                    if pending[0].step():
                        done += 1
                    else:
                        pending.pop(0)

            def defer(group):
                pending.append(group)
                return group

            # upfront work: projections for chunks 0/1 + V strips 0,1
            kv_groups = {0: Group(kv_proj_pieces(0))}
            kv_groups[0].finish()
            q_groups = {0: Group(q_proj_pieces(0))}
            q_groups[0].finish()
            q_groups[1] = defer(Group(q_proj_pieces(1)))
            v_transpose(0)
            v_transpose(1)

            for c in range(NCHUNK):
                # force-complete everything this chunk consumes
                ckv = c // 2
                if ckv in kv_groups:
                    kv_groups[ckv].finish()
                if c in q_groups:
                    q_groups[c].finish()
                # the proj PSUM pool has 2 bufs: all deferred groups must be
                # complete before v_transpose tiles rotate the pool
                while pending:
                    drain(1)
                if c > 0:
                    # V strips 2c, 2c+1 are first used by this chunk's
                    # diagonal PV pair (kv chunk c//2 just ensured)
                    v_transpose(2 * c)
                    v_transpose(2 * c + 1)
                # schedule next-chunk work as filler for this chunk
                if c + 1 < NCHUNK and (c + 1) % 2 == 0:
                    kv_groups[(c + 1) // 2] = defer(
                        Group(kv_proj_pieces((c + 1) // 2))
                    )
                if c + 2 < NCHUNK:
                    q_groups[c + 2] = defer(Group(q_proj_pieces(c + 2)))

                # ---- attention: chunk c attends local strips 0..2c+1 ----
                npair = c + 1
                ps_o = pso_pool.tile([H + 1, CHUNK], dt_f32, tag="pso")
                prev_p = None
                prev_i = -1

                def pv_pair(i, p_sb, diag):
                    l0 = 2 * i
                    nc.tensor.matmul(
                        ps_o,
                        lhsT=v_nat[:, l0 * VSTRIDE : l0 * VSTRIDE + 65],
                        rhs=p_sb[:, 0:CHUNK],
                        start=(i == 0),
                        stop=False,
                    )
                    if diag:
                        # second diagonal strip: only cols 256:512 are live
                        nc.tensor.matmul(
                            ps_o[:, CHUNK // 2 : CHUNK],
                            lhsT=v_nat[:, (l0 + 1) * VSTRIDE : (l0 + 1) * VSTRIDE + 65],
                            rhs=p_sb[:, CHUNK + CHUNK // 2 : 2 * CHUNK],
                            start=False,
                            stop=True,
                        )
                    else:
                        nc.tensor.matmul(
                            ps_o,
                            lhsT=v_nat[:, (l0 + 1) * VSTRIDE : (l0 + 1) * VSTRIDE + 65],
                            rhs=p_sb[:, CHUNK : 2 * CHUNK],
                            start=False,
                            stop=False,
                        )

                for i in range(npair):
                    l0, l1 = 2 * i, 2 * i + 1
                    ps_s = pss_pool.tile([128, 2 * CHUNK], dt_f32, tag="pss")
                    # row-tiled pair: strip l0 in array rows 0:63,
                    # strip l1 in rows 64:127, concurrent
                    nc.tensor.matmul(
                        ps_s[:, 0:CHUNK],
                        lhsT=kt_ap(l0),
                        rhs=q_tiles[c][0:64, :],
                        start=True,
                        stop=True,
                    )
                    if i == npair - 1:
                        # diagonal pair: only cols 256:512 of strip l1 are
                        # live downstream (rest fully causal-masked)
                        nc.tensor.matmul(
                            ps_s[:, CHUNK + CHUNK // 2 : 2 * CHUNK],
                            lhsT=kt_ap(l1),
                            rhs=q_tiles[c][64:128, CHUNK // 2 : CHUNK],
                            start=True,
                            stop=True,
                        )
                    else:
                        nc.tensor.matmul(
                            ps_s[:, CHUNK : 2 * CHUNK],
                            lhsT=kt_ap(l1),
                            rhs=q_tiles[c][64:128, :],
                            start=True,
                            stop=True,
                        )
                    p_sb = ppool.tile([128, 2 * CHUNK], dt_bf, tag="p")
                    nc.scalar.activation(
                        p_sb,
                        ps_s,
                        mybir.ActivationFunctionType.Exp,
                        scale=scale,
                    )
                    if i == npair - 1:
                        # causal mask on the diagonal pair, split into the
                        # two live regions (cols 512:768 are never read):
                        # the A-half PV can start as soon as its mask lands
                        nc.vector.tensor_mul(
                            p_sb[:, 0:CHUNK], p_sb[:, 0:CHUNK],
                            masks_sb[:, 0:CHUNK],
                        )
                        nc.vector.tensor_mul(
                            p_sb[:, CHUNK + CHUNK // 2 : 2 * CHUNK],
                            p_sb[:, CHUNK + CHUNK // 2 : 2 * CHUNK],
                            masks_sb[:, CHUNK + CHUNK // 2 : 2 * CHUNK],
                        )
                    drain(3)
                    if prev_p is not None:
                        pv_pair(prev_i, prev_p, diag=False)
                    prev_p, prev_i = p_sb, i
                pv_pair(prev_i, prev_p, diag=True)

                o_sb = opool.tile([H + 1, CHUNK], dt_f32, tag="o")
                nc.vector.tensor_copy(o_sb, ps_o)
                nc.sync.dma_start(
                    out_d.ap()[:, c * CHUNK : (c + 1) * CHUNK], o_sb
                )
            while pending:
                drain(1)
            if DBG:
                nc.sync.dma_start(dbg_v.ap(), v_nat)

    nc.compile()
    return nc


def _perm(rho):
    """Rotated-order permutation: rotated position i holds original token
    perm[i]. Involutive (half swap within each 256-block)."""
    i = np.arange(T)
    return (i // 256) * 256 + ((i % 256) + 128 * rho) % 256


def _make_in_maps(x, Wq, bq, Wk, bk, Wv, bv):
    wq_pack = np.ascontiguousarray(
        np.concatenate([Wq.reshape(4, 128, 64)] * 2, axis=2)
        .transpose(1, 0, 2)
        .reshape(128, 512)
    ).astype(bf16)
    wkva_pack = np.ascontiguousarray(
        np.concatenate([Wk.reshape(4, 128, 64), Wv.reshape(4, 128, 64)], axis=2)
        .transpose(1, 0, 2)
        .reshape(128, 512)
    ).astype(bf16)
    wkvb_pack = np.ascontiguousarray(
        np.concatenate([Wv.reshape(4, 128, 64), Wk.reshape(4, 128, 64)], axis=2)
        .transpose(1, 0, 2)
        .reshape(128, 512)
    ).astype(bf16)
    bias_q = np.ascontiguousarray(
        np.concatenate([bq, bq])[:, None]
    ).astype(np.float32)
    bias_kv = np.ascontiguousarray(
        np.stack([np.concatenate([bk, bv]), np.concatenate([bv, bk])], axis=1)
    ).astype(np.float32)

    kk = np.arange(128)[:, None]
    in_maps = []
    for b in range(B):
        xt_b = np.ascontiguousarray(x[b].T).astype(bf16).reshape(4, 128, T)
        for rho in range(2):
            perm = _perm(rho)
            xt_rot = xt_b[:, :, perm]  # rotated token order
            xt_in = np.ascontiguousarray(
                xt_rot.reshape(4, 128, 4, T // 4).transpose(2, 1, 0, 3)
            )
            # masks: columns are in rotated order; v = original
            # within-chunk offset of rotated column jcol (chunk-indep.)
            v = perm[:CHUNK]
            m0 = (kk - v[None, :] <= -128 * rho).astype(bf16)
            m1 = (kk - v[None, :] <= -256 - 128 * rho).astype(bf16)
            masks_np = np.ascontiguousarray(np.concatenate([m0, m1], axis=1))
            in_maps.append(
                {
                    "xt": xt_in,
                    "wq": wq_pack,
                    "wkva": wkva_pack,
                    "wkvb": wkvb_pack,
                    "bias_q": bias_q,
                    "bias_kv": bias_kv,
                    "masks": masks_np,
                }
            )
    return in_maps


def _combine(results):
    out = np.empty((B, T, H), np.float32)
    p1 = _perm(1)
    for b in range(B):
        a0 = results[2 * b]["out"].astype(np.float64)
        a1 = results[2 * b + 1]["out"].astype(np.float64)
        a1 = a1[:, p1]  # un-rotate core-1 columns (involutive perm)
        num = a0[:H] + a1[:H]
        den = a0[H] + a1[H]
        out[b] = (num / den).T.astype(np.float32)
    return out


def _run(trace=False, **inputs):
    from concourse import bass_utils

    nc = _build()
    in_maps = _make_in_maps(
        np.asarray(inputs["x"], np.float32),
        np.asarray(inputs["Wq"], np.float32),
        np.asarray(inputs["bq"], np.float32),
        np.asarray(inputs["Wk"], np.float32),
        np.asarray(inputs["bk"], np.float32),
        np.asarray(inputs["Wv"], np.float32),
        np.asarray(inputs["bv"], np.float32),
    )
    res = bass_utils.run_bass_kernel_spmd(
        nc, in_maps, list(range(NCORES)), trace=trace
    )
    return _combine(res.results), res.exec_time_ns


def kernel(**inputs):
    out, _ = _run(trace=False, **inputs)
    return out
